# revision 22
# baseline (speedup 1.0000x reference)
"""Trainium2 Bass kernel for a pre-LN transformer block (B=128,T=256,C=384,H=6,D=64).

Data-parallel over batch across 8 NeuronCores (16 batches/core), processed in
pairs so QKV and FFN1 stream a 512-wide moving operand. All matmuls are
float32r (self-loading stationary, 1 cycle/row at >=256 moving). LN gamma/beta
fold into the weights on the host; device LN is (x - mu) * rstd via
bn_stats/bn_aggr, with the normalize apply on gpsimd to offload DVE.
Attention uses transposed scores ([S,T]): causal masking adds -BIG*(s-t) on
the PE via one 512-wide tril matmul per head; softmax denominators come from
M=64 all-ones matmuls that land both heads of a pair in the upper/lower
partition halves of one PSUM bank (tile_position), so normalization is one
packed reciprocal + one packed multiply per head pair; attn@v packs head
pairs the same way, letting the output projection contract full 128-row hd
chunks (3 matmuls per token chunk instead of 6). The FFN runs FFN1 -> relu
(ACT, bias fused) -> output-stationary FFN2, and stages are software
pipelined: the next pair's DMA/LN1/QKV is emitted between FFN1 and FFN2 so
the PE never waits on the relu evacuation.
"""

import sys

if "/opt/trn_rl_repo" not in sys.path:
    sys.path.insert(0, "/opt/trn_rl_repo")

import numpy as np

import concourse.bass as bass
import concourse.mybir as mybir
import concourse.tile as tile
from concourse import bacc

# All ACT functions used here (Exp, Ln, Relu, Identity, Copy) live in the
# 'natural_log_exp_and_others' table set. Blank the other sets (preserving
# dict order, which defines act_func_set_id) so the table-load fixpoint
# settles on a single ACT_TABLE_LOAD instead of thrashing sets per batch.
_KEEP_ACT_SET = "natural_log_exp_and_others"
_orig_get_act_tables = bacc.get_activation_tables


def _one_set_tables(arch):
    t = _orig_get_act_tables(arch)
    assert _KEEP_ACT_SET in t
    return {k: (v if k == _KEEP_ACT_SET else set()) for k, v in t.items()}


bacc.get_activation_tables = _one_set_tables

F32 = mybir.dt.float32
F32R = mybir.dt.float32r
F8 = mybir.dt.float8e4
DR = mybir.MatmulPerfMode.DoubleRow
AF = mybir.ActivationFunctionType
ALU = mybir.AluOpType

B, T, C, H, D = 128, 256, 384, 6, 64
NCORES = 8
BL = B // NCORES          # batches per core
F = 4 * C                 # 1536
P = 128
TCH = T // P              # 2 token chunks
CCH = C // P              # 3 channel chunks
FCH = F // P              # 12 ffn chunks
HD = H * D                # 384
NPC = H // 2              # head pairs
SCALE = float(C) ** -0.5  # reference scales by full model dim
EPS = 1e-5


def build_program(bl=BL, flags=frozenset(), repeat=1, **_compat):
    """Per-core Bass program. `flags` lists nonzero bias terms
    ('qb','kb','vb','bo','b1','b2'). `repeat` wraps the whole computation in
    a hardware loop (benchmarking only)."""
    assert bl % 2 == 0
    use_qb = "qb" in flags
    use_kb = "kb" in flags
    use_vb = "vb" in flags
    use_bo = "bo" in flags
    use_b1 = "b1" in flags
    use_b2 = "b2" in flags
    use_ones_row = use_vb or use_bo or use_b2

    nc = bacc.Bacc("TRN2", target_bir_lowering=False, debug=False,
                   num_devices=NCORES)

    x_d = nc.dram_tensor("x", [bl, T, C], F32, kind="ExternalInput")
    wq_d = nc.dram_tensor("wq", [P, CCH, HD], F8, kind="ExternalInput")
    wk_d = nc.dram_tensor("wk", [P, CCH, HD], F8, kind="ExternalInput")
    wv_d = nc.dram_tensor("wv", [P, CCH, HD], F8, kind="ExternalInput")
    qb_d = nc.dram_tensor("qb", [P, CCH], F32, kind="ExternalInput")
    kb_d = nc.dram_tensor("kb", [P, CCH], F32, kind="ExternalInput")
    vb_d = nc.dram_tensor("vb", [1, HD], F32R, kind="ExternalInput")
    wo_d = nc.dram_tensor("wo2", [P, NPC, C], F32R, kind="ExternalInput")
    bo_d = nc.dram_tensor("bo", [1, C], F32R, kind="ExternalInput")
    w1_d = nc.dram_tensor("w1", [P, CCH, F], F8, kind="ExternalInput")
    b1_d = nc.dram_tensor("b1c", [P, FCH], F32, kind="ExternalInput")
    w2_d = nc.dram_tensor("w2", [P, FCH, C], F8, kind="ExternalInput")
    b2_d = nc.dram_tensor("b2", [1, C], F32R, kind="ExternalInput")
    id_d = nc.dram_tensor("ident", [P, P], F32R, kind="ExternalInput")
    on_d = nc.dram_tensor("ones64", [P, 64], F32R, kind="ExternalInput")
    tl_d = nc.dram_tensor("trilm", [P, P], F32R, kind="ExternalInput")
    ng_d = nc.dram_tensor("negm", [P, TCH, T], F32R, kind="ExternalInput")
    or_d = nc.dram_tensor("onesrow", [1, P], F32R, kind="ExternalInput")
    y_d = nc.dram_tensor("y", [bl, T, C], F32, kind="ExternalOutput")

    with tile.TileContext(nc) as tc:
        with (
            tc.tile_pool(name="wpool", bufs=1) as wp,
            tc.tile_pool(name="work", bufs=2) as wk_pool,
            tc.tile_pool(name="big", bufs=1) as bigp,
            tc.tile_pool(name="ps", bufs=8, space="PSUM") as ps,
        ):
            # ---- load weights/constants once ----
            wq = wp.tile([P, CCH, HD], F8)
            wkk = wp.tile([P, CCH, HD], F8)
            wv = wp.tile([P, CCH, HD], F8)
            wo2 = wp.tile([P, NPC, C], F32R)
            w1 = wp.tile([P, CCH, F], F8)
            w2 = wp.tile([P, FCH, C], F8)
            ident = wp.tile([P, P], F32R)
            ones64 = wp.tile([P, 64], F32R)
            trilm = wp.tile([P, P], F32R)
            negm = wp.tile([P, TCH, T], F32R)
            epsb = wp.tile([P, 1], F32)
            nc.gpsimd.memset(epsb[:], EPS)
            nc.sync.dma_start(wq[:], wq_d[:])
            nc.sync.dma_start(wkk[:], wk_d[:])
            nc.sync.dma_start(wv[:], wv_d[:])
            nc.sync.dma_start(wo2[:], wo_d[:])
            nc.sync.dma_start(w1[:], w1_d[:])
            nc.sync.dma_start(w2[:], w2_d[:])
            nc.sync.dma_start(ident[:], id_d[:])
            nc.sync.dma_start(ones64[:], on_d[:])
            nc.sync.dma_start(trilm[:], tl_d[:])
            nc.sync.dma_start(negm[:], ng_d[:])
            qb = kb = vb = bo = b1c = b2 = ones_row = None
            if use_ones_row:
                ones_row = wp.tile([1, P], F32R)
                nc.sync.dma_start(ones_row[:], or_d[:])
            if use_qb:
                qb = wp.tile([P, CCH], F32)
                nc.sync.dma_start(qb[:], qb_d[:])
            if use_kb:
                kb = wp.tile([P, CCH], F32)
                nc.sync.dma_start(kb[:], kb_d[:])
            if use_vb:
                vb = wp.tile([1, HD], F32R)
                nc.sync.dma_start(vb[:], vb_d[:])
            if use_bo:
                bo = wp.tile([1, C], F32R)
                nc.sync.dma_start(bo[:], bo_d[:])
            if use_b1:
                b1c = wp.tile([P, FCH], F32)
                nc.sync.dma_start(b1c[:], b1_d[:])
            if use_b2:
                b2 = wp.tile([1, C], F32R)
                nc.sync.dma_start(b2[:], b2_d[:])

            def layer_norm_T(src, dstT, i, ln, evac_act):
                """src: [P, TCH, C] tokens-major f32. Writes (src-mu)*rstd
                transposed into dstT[:, :, i, :] ([P, CCH, 2, T])."""
                st6 = wk_pool.tile([P, TCH, 6], F32, tag=f"st6_{i}")
                mv = wk_pool.tile([P, TCH, 2], F32, tag=f"mv_{i}")
                rstd = wk_pool.tile([P, TCH], F32, tag=f"rstd_{i}")
                for tch in range(TCH):
                    nc.vector.bn_stats(st6[:, tch, :], src[:, tch, :])
                    nc.vector.bn_aggr(mv[:, tch, :], st6[:, tch, :])
                # rstd = exp(-0.5 * ln(var + eps))
                nc.scalar.activation(rstd[:], mv[:, :, 1], AF.Ln, bias=epsb[:])
                nc.scalar.activation(rstd[:], rstd[:], AF.Exp, scale=-0.5)
                xn = wk_pool.tile([P, TCH, C], F32R, tag=f"xn_{ln}_{i}",
                                  bufs=1)
                # LN1 normalize on gpsimd (off critical path); LN2 on DVE
                # (it sits on the B-tail serial chain)
                eng = nc.vector if ln == 1 else nc.gpsimd
                for tch in range(TCH):
                    eng.tensor_scalar(
                        xn[:, tch, :], src[:, tch, :],
                        mv[:, tch, 0:1], rstd[:, tch:tch + 1],
                        ALU.subtract, ALU.mult,
                    )
                tr = ps.tile([P, 2, T], F32R, tag="ps", name=f"tr_{ln}_{i}")
                trb = ps.tile([P, T], F32R, tag="ps", name=f"trb_{ln}_{i}")
                for tch in range(TCH):
                    for cc in range(CCH):
                        dst = (trb[:, tch * P:(tch + 1) * P] if cc == 2 else
                               tr[:, cc, tch * P:(tch + 1) * P])
                        nc.tensor.transpose(
                            dst, xn[:, tch, cc * P:(cc + 1) * P], ident[:])
                del evac_act
                nc.scalar.copy(dstT[:, 0:2, i, :], tr[:])
                nc.scalar.copy(dstT[:, 2, i, :], trb[:])

            def stage_A1(pb):
                """DMA x + LN1 for pair pb."""
                st = {"bp": (2 * pb, 2 * pb + 1)}
                xts = st["xts"] = []
                xnT2 = st["xnT2"] = wk_pool.tile(
                    [P, CCH, 2, T], F8, tag="xnT2", name="xnT2", bufs=1)
                for i, b in enumerate(st["bp"]):
                    xt = wk_pool.tile([P, TCH, C], F32, tag=f"xt{i}")
                    nc.sync.dma_start(
                        xt[:], x_d[b].rearrange("(tc p) c -> p tc c", p=P))
                    xts.append(xt)
                    layer_norm_T(xt, xnT2, i, 0, evac_act=(i == 0))
                return st

            def stage_A2(st):
                """QKV + V projections; q,k transposed [hd, (b,t)]."""
                xnT2 = st["xnT2"]
                qsb2 = st["qsb2"] = wk_pool.tile(
                    [P, CCH, 2, T], F32R, tag="qsb2", name="qsb2")
                ksb2 = st["ksb2"] = wk_pool.tile(
                    [P, CCH, 2, T], F32R, tag="ksb2", name="ksb2")
                for wmat, bias_t, use_b, dst, eng in (
                    (wq, qb, use_qb, qsb2, "act"),
                    (wkk, kb, use_kb, ksb2, "dve"),
                ):
                    for mc in range(CCH):
                        pp = ps.tile([P, 2, T], F32, tag="ps", name="pp")
                        nc.tensor.matmul(
                            pp[:, :, :],
                            wmat[:, 0:2, mc * P:(mc + 1) * P],
                            xnT2[:, 0:2, :, :],
                            start=True, stop=False, perf_mode=DR)
                        nc.tensor.matmul(
                            pp[:, :, :],
                            wmat[:, 2, mc * P:(mc + 1) * P],
                            xnT2[:, 2, :, :],
                            start=False, stop=True)
                        if use_b:
                            nc.scalar.activation(
                                dst[:, mc, :, :], pp[:], AF.Identity,
                                bias=bias_t[:, mc:mc + 1])
                        elif eng == "act":
                            nc.scalar.copy(dst[:, mc, :, :], pp[:])
                        else:
                            nc.vector.tensor_copy(dst[:, mc, :, :], pp[:])

                # v natural [s, hd]
                vsbs = st["vsbs"] = []
                for i in range(2):
                    vsb = wk_pool.tile([P, TCH, HD], F32R, tag=f"vsb{i}")
                    vsbs.append(vsb)
                    for sc in range(TCH):
                        vp = ps.tile([P, HD], F32, tag="ps", name="vp")
                        nc.tensor.matmul(
                            vp[:, :],
                            xnT2[:, 0:2, i, sc * P:(sc + 1) * P],
                            wv[:, 0:2, :],
                            start=True, stop=False, perf_mode=DR)
                        nc.tensor.matmul(
                            vp[:, :],
                            xnT2[:, 2, i, sc * P:(sc + 1) * P],
                            wv[:, 2, :],
                            start=False, stop=(not use_vb),
                        )
                        if use_vb:
                            nc.tensor.matmul(
                                vp[:, :], ones_row[0:1, :], vb[0:1, :],
                                start=False, stop=True)
                        if sc == 0:
                            nc.scalar.copy(vsb[:, sc, :], vp[:])
                        else:
                            nc.vector.tensor_copy(vsb[:, sc, :], vp[:])

            def stage_B(st):
                """Attention + output projection + LN2 for the pair."""
                qsb2, ksb2, vsbs, xts = (st["qsb2"], st["ksb2"], st["vsbs"],
                                         st["xts"])
                xnews = st["xnews"] = []
                for i in range(2):
                    e_all = bigp.tile([P, TCH, H, T], F32R, tag="e_all",
                                      bufs=2)
                    for h in range(H):
                        hc, ho = h // 2, 64 * (h % 2)
                        ksl = ksb2[ho:ho + D, hc, i, :]
                        qsl = qsb2[ho:ho + D, hc, i, :]
                        sp = ps.tile([P, TCH, T], F32, tag="ps", name="sp")
                        for sc in range(TCH):
                            nc.tensor.matmul(
                                sp[:, sc, :], ksl[:, sc * P:(sc + 1) * P],
                                qsl[:, :], start=True, stop=True)
                        nc.scalar.activation(
                            e_all[:, :, h, :], sp[:], AF.Exp, scale=SCALE)
                        # causal mask on gpsimd: zero e where s > t
                        # (affine = t - 128*sc - p, keep where >= 0)
                        nc.gpsimd.affine_select(
                            e_all[:, :, h, :], e_all[:, :, h, :],
                            pattern=[[-128, 2], [1, 256]],
                            compare_op=ALU.is_ge, fill=0.0,
                            base=0, channel_multiplier=-1)

                    osb = wk_pool.tile([P, NPC, T], F32R, tag="osb", bufs=2)
                    for pc in range(NPC):
                        # softmax denominators (M=64 all-ones matmuls) and
                        # attn@v, column-packed per head pair; the multiply
                        # relocates head 1 into osb's upper partitions (DVE
                        # handles in/out partition-offset mismatch)
                        dpc = ps.tile([64, 2, T], F32, tag="ps", name="dpc")
                        opc = ps.tile([64, 2, T], F32, tag="ps", name="opc")
                        rec = wk_pool.tile([64, 2, T], F32, tag="rec", bufs=3)
                        for j in range(2):
                            h = 2 * pc + j
                            for sc in range(TCH):
                                nc.tensor.matmul(
                                    dpc[:, j, :], ones64[:],
                                    e_all[:, sc, h, :],
                                    start=(sc == 0), stop=(sc == TCH - 1))
                                nc.tensor.matmul(
                                    opc[:, j, :],
                                    vsbs[i][:, sc, h * D:(h + 1) * D],
                                    e_all[:, sc, h, :],
                                    start=(sc == 0), stop=(sc == TCH - 1))
                        nc.vector.reciprocal(rec[:], dpc[:])
                        for j in range(2):
                            nc.vector.tensor_tensor(
                                osb[64 * j:64 * (j + 1), pc, :],
                                opc[:, j, :], rec[:, j, :], ALU.mult)

                    # out proj + residual; osb rows are (j,d) packed to
                    # match wo2's 128-row hd chunks
                    xnew = wk_pool.tile([P, TCH, C], F32, tag=f"xnew{i}")
                    xnews.append(xnew)
                    for tcc in range(TCH):
                        ap_t = ps.tile([P, C], F32, tag="ps", name="ap_t")
                        for pc in range(NPC):
                            nc.tensor.matmul(
                                ap_t[:, :],
                                osb[:, pc, tcc * P:(tcc + 1) * P],
                                wo2[:, pc, :],
                                start=(pc == 0),
                                stop=(pc == NPC - 1 and not use_bo))
                        if use_bo:
                            nc.tensor.matmul(
                                ap_t[:, :], ones_row[0:1, :], bo[0:1, :],
                                start=False, stop=True)
                        nc.vector.tensor_tensor(
                            xnew[:, tcc, :], ap_t[:, :], xts[i][:, tcc, :],
                            ALU.add)

            def stage_L2(st):
                """LN2 for both batches -> xn2T2."""
                xn2T2 = st["xn2T2"] = wk_pool.tile(
                    [P, CCH, 2, T], F8, tag="xn2T2", name="xn2T2", bufs=1)
                for i in range(2):
                    layer_norm_T(st["xnews"][i], xn2T2, i, 1,
                                 evac_act=(i == 1))

            def stage_C(st):
                """FFN1 + bias/relu (ACT) -> hsm."""
                hsm = st["hsm"] = wk_pool.tile(
                    [P, FCH, 2 * T], F8, tag="hsm", name="hsm", bufs=1)
                xn2T2 = st["xn2T2"]
                for mo in range(FCH):
                    hp = ps.tile([P, 2, T], F32, tag="ps", name="hp")
                    nc.tensor.matmul(
                        hp[:, :, :],
                        w1[:, 0:2, mo * P:(mo + 1) * P],
                        xn2T2[:, 0:2, :, :],
                        start=True, stop=False, perf_mode=DR)
                    nc.tensor.matmul(
                        hp[:, :, :],
                        w1[:, 2, mo * P:(mo + 1) * P],
                        xn2T2[:, 2, :, :],
                        start=False, stop=True)
                    if use_b1:
                        nc.scalar.activation(
                            hsm[:, mo, :], hp[:, :, :], AF.Relu,
                            bias=b1c[:, mo:mo + 1])
                    else:
                        nc.scalar.activation(hsm[:, mo, :], hp[:, :, :],
                                             AF.Relu)

            def stage_D_mm(st, i):
                """Output-stationary FFN2 matmuls for batch i."""
                hsm = st["hsm"]
                fps = st.setdefault("fps", {})
                for tcc in range(TCH):
                    j = 2 * i + tcc
                    fp = ps.tile([P, C], F32, tag="ps", name="fp")
                    fps[j] = fp
                    for m in range(FCH // 2):
                        nc.tensor.matmul(
                            fp[:, :],
                            hsm[:, 2 * m:2 * m + 2, j * P:(j + 1) * P],
                            w2[:, 2 * m:2 * m + 2, :],
                            start=(m == 0),
                            stop=(m == FCH // 2 - 1 and not use_b2),
                            perf_mode=DR)
                    if use_b2:
                        nc.tensor.matmul(
                            fp[:, :], ones_row[0:1, :], b2[0:1, :],
                            start=False, stop=True)

            def stage_D_evac(st, i):
                """FFN2 residual add + DMA out for batch i."""
                xnews, bp, fps = st["xnews"], st["bp"], st["fps"]
                yout = wk_pool.tile([P, TCH, C], F32, tag=f"yout{i}")
                for tcc in range(TCH):
                    nc.vector.tensor_tensor(
                        yout[:, tcc, :], fps[2 * i + tcc][:, :],
                        xnews[i][:, tcc, :], ALU.add)
                nc.sync.dma_start(
                    y_d[bp[i]].rearrange("(tc p) c -> p tc c", p=P),
                    yout[:])

            def body():
                npairs = bl // 2
                st = stage_A1(0)
                stage_A2(st)
                prev = None
                for pb in range(npairs):
                    if prev is not None:
                        stage_D_mm(prev, 0)
                        stage_D_evac(prev, 0)
                    stage_B(st)
                    if prev is not None:
                        stage_D_mm(prev, 1)
                    nxt = None
                    if pb + 1 < npairs:
                        nxt = stage_A1(pb + 1)
                        stage_A2(nxt)
                    stage_L2(st)
                    if prev is not None:
                        stage_D_evac(prev, 1)
                    stage_C(st)
                    prev, st = st, nxt
                for i in range(2):
                    stage_D_mm(prev, i)
                    stage_D_evac(prev, i)

            if repeat > 1:
                with tc.For_i(0, repeat, 1):
                    body()
            else:
                body()

    nc.compile()
    return nc


def _make_negm():
    # negm[j, sc, t] moving operand; with trilm (lhsT[j, s] = 1 iff j <= s)
    # the accumulated matmul adds -BIG * #{j: j <= s_blk and cond(j, t)},
    # nonzero exactly where global s > t.
    BIG = np.float32(1e30)
    f32 = np.float32
    m = np.zeros((P, TCH, T), dtype=f32)
    jgt = np.tril(np.ones((P, P), dtype=f32), -1)  # [j, t] = 1 iff j > t
    m[:, 0, 0:P] = -BIG * jgt          # diagonal block of s-chunk 0
    m[:, 1, 0:P] = -BIG                # s-chunk 1 vs t-chunk 0: all masked
    m[:, 1, P:2 * P] = -BIG * jgt      # diagonal block of s-chunk 1
    return m


def prep_weights(Wq, Wk, Wv, Wo, bo, W1, b1, W2, b2, g1, be1, g2, be2):
    """Fold LN gamma/beta into projection weights; rearrange to SBUF layouts."""
    f32 = np.float32
    import ml_dtypes
    f8 = ml_dtypes.float8_e4m3

    def kchunk(w, kdim):  # [K, M] -> [P, K//P, M]
        m = w.shape[1]
        return np.ascontiguousarray(
            w.reshape(kdim // P, P, m).transpose(1, 0, 2)).astype(f32)

    Wq2 = Wq.transpose(1, 0, 2).reshape(C, HD)
    Wk2 = Wk.transpose(1, 0, 2).reshape(C, HD)
    Wv2 = Wv.transpose(1, 0, 2).reshape(C, HD)
    out = {
        "wq": kchunk(g1[:, None] * Wq2, C).astype(f8),
        "wk": kchunk(g1[:, None] * Wk2, C).astype(f8),
        "wv": kchunk(g1[:, None] * Wv2, C).astype(f8),
        # [128, NPC, C]: row p of chunk pc is Wo[128*pc + p]
        "wo2": np.ascontiguousarray(
            Wo.reshape(NPC, P, C).transpose(1, 0, 2)).astype(f32),
        "w1": kchunk(g2[:, None] * W1, C).astype(f8),
        "w2": kchunk(W2, F).astype(f8),
        "ident": np.eye(P, dtype=f32),
        "ones64": np.ones((P, 64), dtype=f32),
        "trilm": np.tril(np.ones((P, P), dtype=f32)).T.copy(),
        "negm": _make_negm(),
        "onesrow": np.ones((1, P), dtype=f32),
    }
    qb = be1 @ Wq2
    kb = be1 @ Wk2
    vb = be1 @ Wv2
    b1e = be2 @ W1 + b1
    out["qb"] = np.ascontiguousarray(qb.reshape(CCH, P).T).astype(f32)
    out["kb"] = np.ascontiguousarray(kb.reshape(CCH, P).T).astype(f32)
    out["vb"] = vb[None, :].astype(f32)
    out["bo"] = bo[None, :].astype(f32)
    out["b1c"] = np.ascontiguousarray(b1e.reshape(FCH, P).T).astype(f32)
    out["b2"] = b2[None, :].astype(f32)
    flags = set()
    for name, vec in (("qb", qb), ("kb", kb), ("vb", vb),
                      ("bo", bo), ("b1", b1e), ("b2", b2)):
        if np.any(vec != 0):
            flags.add(name)
    return out, frozenset(flags)


_PROGRAM_CACHE = {}


def _get_program(bl, flags):
    key = (bl, flags)
    if key not in _PROGRAM_CACHE:
        _PROGRAM_CACHE[key] = build_program(bl, flags)
    return _PROGRAM_CACHE[key]


def kernel(x, Wq, Wk, Wv, Wo, bo, W1, b1, W2, b2, g1, be1, g2, be2, **kw):
    from concourse.bass_utils import run_bass_kernel_spmd

    args = [np.asarray(a, dtype=np.float32) for a in
            (x, Wq, Wk, Wv, Wo, bo, W1, b1, W2, b2, g1, be1, g2, be2)]
    x = args[0]
    wmap, flags = prep_weights(*args[1:])
    nc = _get_program(BL, flags)
    xs = x.reshape(NCORES, BL, T, C)
    in_maps = []
    for c in range(NCORES):
        m = {"x": np.ascontiguousarray(xs[c])}
        m.update(wmap)
        in_maps.append(m)
    res = run_bass_kernel_spmd(nc, in_maps, list(range(NCORES)), **kw)
    global _last_results
    _last_results = res
    y = np.stack([res.results[i]["y"] for i in range(NCORES)], axis=0)
    return y.reshape(B, T, C)


_last_results = None
